# revision 30
# baseline (speedup 1.0000x reference)
"""Trainium2 Bass kernel for the MDL_RNN_mnist spiking network.

Strategy (data-parallel over batch, 8 NeuronCores, B_local = 64):
  - Spikes are Bernoulli draws from jax's threefry PRNG; they are generated on
    host (bit-exact with the reference) and streamed to the device as bf16 0/1.
  - Input currents Iin are precomputed on-device with one big batched matmul
    (spk @ Win.T over all 100 steps), stored to DRAM in bf16 and streamed back
    during the scan.
  - The recurrent matmul r @ Wr.T uses the rank-256 factorization
    Wr = (l*pin) @ pout.T, so each step is two thin matmuls through the
    P=256 bottleneck instead of a 2048x2048 product (4x fewer MACs).
  - Refractory bookkeeping uses a countdown counter C (exactly equivalent to
    the reference's tlast comparison for DT=1, TREF=2), which folds the
    refractory+reset masks into a single fused DVE op.
  - All decay constants are folded into the weights on host; h and r are kept
    pre-scaled by 1/c (c = DT/(TAUD*TAUR)) so their updates are single fused
    scalar_tensor_tensor ops; c is folded into pout and Wout.
  - Membrane state stays fp32 (spike thresholding is chaos-sensitive); weights
    and binary states are bf16.
Final softmax (512x10) runs on host in fp32, matching the reference formula.
"""

import numpy as np
import ml_dtypes

T, DT, TAUM, TAUD, TAUR, VTHR, TREF = 100, 1.0, 10.0, 30.0, 2.0, 1.0, 2.0
B, IN, H, P, OUT = 512, 784, 2048, 256, 10
NCORES = 8
BL = B // NCORES          # 64 batch per core
HT, PT, KT = H // 128, P // 128, 7   # 16, 2, 7 (IN padded 784 -> 896)
INP = KT * 128            # 896
BF16 = ml_dtypes.bfloat16

_CACHE = {}

# engine knobs (walrus-legal only): eng_s/eng_C: "v"=DVE, "g"=GPSIMD;
# eng_D: "a"=ScalarE, "g"=GPSIMD.  All-DVE won on hardware (GPSIMD shares
# the DVE SBUF port; measured 41.7 vs 72.8 us/step).
_CFG = dict(eng_s="v", eng_C="v", eng_D="a",
            x_bufs=2, u_bufs=2, pp_bufs=2, nsteps=T)


def _decay_consts():
    lm = np.float32(np.exp(np.float32(-DT / TAUM)))
    ld = np.float32(np.exp(np.float32(-DT / TAUD)))
    lr = np.float32(np.exp(np.float32(-DT / TAUR)))
    c = np.float32(DT / (TAUD * TAUR))
    return lm, ld, lr, c


def _build_nc():
    import concourse.bacc as bacc
    import concourse.mybir as mybir
    import concourse.tile as tile

    dt = mybir.dt
    A = mybir.AluOpType
    AF = mybir.ActivationFunctionType
    lm, ld, lr, _c = _decay_consts()
    lm, ld, lr = float(lm), float(ld), float(lr)

    nc = bacc.Bacc("TRN2", target_bir_lowering=False, debug=False,
                   enable_asserts=False, num_devices=NCORES)
    cfg = _CFG

    spk_d = nc.dram_tensor("spk", [KT, 128, T, BL], dt.bfloat16,
                           kind="ExternalInput").ap()
    poutT_d = nc.dram_tensor("poutT", [128, HT, PT, 128], dt.bfloat16,
                             kind="ExternalInput").ap()
    pinT_d = nc.dram_tensor("pinT", [128, PT, HT, 128], dt.bfloat16,
                            kind="ExternalInput").ap()
    winT_d = nc.dram_tensor("winT", [128, KT, HT, 128], dt.bfloat16,
                            kind="ExternalInput").ap()
    iden_d = nc.dram_tensor("iden", [128, 128], dt.bfloat16,
                            kind="ExternalInput").ap()
    woutT_d = nc.dram_tensor("woutT", [128, HT, OUT], dt.bfloat16,
                             kind="ExternalInput").ap()
    rout_d = nc.dram_tensor("rout", [BL, OUT], dt.float32,
                            kind="ExternalOutput").ap()

    def sbuf(name, shape, dtype):
        return nc.alloc_sbuf_tensor(name, list(shape), dtype).ap()

    # persistent state [128, HT, BL]: h index = j*128 + partition.
    # mem must stay fp32 (threshold chaos); hs/rs/rms are bf16 (validated:
    # output error unchanged vs fp32 — bf16 weights dominate the noise).
    mem = sbuf("mem", [128, HT, BL], dt.float32)
    hs = sbuf("hs", [128, HT, BL], dt.bfloat16)
    rms = sbuf("rms", [128, HT, BL], dt.bfloat16)
    # resident weights
    poutT = sbuf("poutT_sb", [128, HT, PT, 128], dt.bfloat16)
    pinT = sbuf("pinT_sb", [128, PT, HT, 128], dt.bfloat16)
    winT = sbuf("winT_sb", [128, KT, HT, 128], dt.bfloat16)
    iden = sbuf("iden_sb", [128, 128], dt.bfloat16)
    woutT = sbuf("woutT_sb", [128, HT, OUT], dt.bfloat16)
    bm1 = sbuf("bm1", [128, 1], dt.float32)   # bias constant -1.0 for Relu(C-1)

    CH = 8  # time-chunk for the Iin prepass (N = CH*BL = 512)
    chunks = [(t0, min(CH, T - t0)) for t0 in range(0, T, CH)]

    with tile.TileContext(nc, trace_sim=False) as tc:
        # load weights
        nc.sync.dma_start(poutT, poutT_d)
        nc.sync.dma_start(pinT, pinT_d)
        nc.sync.dma_start(winT, winT_d)
        nc.sync.dma_start(iden, iden_d)
        nc.sync.dma_start(woutT, woutT_d)
        nc.vector.memset(bm1, -1.0)

        # The Iin prepass (spk @ Win.T, PE-heavy) is interleaved into the scan
        # (DVE-heavy): chunk c+1's currents are computed into an SBUF
        # ping-pong buffer (2 h-tiles per step) while chunk c is consumed.
        CH = 8
        nchunks = len(chunks)

        with tc.tile_pool(name="mema", bufs=2) as mema_pool, \
             tc.tile_pool(name="ubf", bufs=2) as ubf_pool, \
             tc.tile_pool(name="sC", bufs=3) as sC_pool, \
             tc.tile_pool(name="rsp", bufs=2) as rs_pool, \
             tc.tile_pool(name="io", bufs=2) as io_pool, \
             tc.tile_pool(name="sp", bufs=2) as sp_pool, \
             tc.tile_pool(name="pp_ps", bufs=cfg["pp_bufs"], space="PSUM") as pp_pool, \
             tc.tile_pool(name="u_ps", bufs=cfg["u_bufs"], space="PSUM") as u_pool, \
             tc.tile_pool(name="x_ps", bufs=cfg["x_bufs"], space="PSUM") as x_pool:

            def emit_sp_dma(ci):
                t0, ch = chunks[ci]
                sp = sp_pool.tile([128, KT, CH, BL], dt.bfloat16, tag="sp")
                nc.sync.dma_start(
                    sp[:, :, :ch, :],
                    spk_d[:, :, t0:t0 + ch, :].rearrange("k p t b -> p k t b"))
                return sp

            def emit_prepass(sp, io, ci, ih):
                ch = chunks[ci][1]
                pp = pp_pool.tile([128, CH * BL], dt.float32, tag="pp")
                for k in range(KT):
                    nc.tensor.matmul(pp[:, :ch * BL], winT[:, k, ih, :],
                                     sp[:, k, :ch, :],
                                     start=(k == 0), stop=(k == KT - 1))
                nc.scalar.activation(io[:, :ch, ih, :], pp[:, :ch * BL], AF.Copy)

            nc.vector.memset(mem, 0.0)
            nc.vector.memset(hs, 0.0)
            nc.vector.memset(rms, 0.0)
            s_prev = sC_pool.tile([128, HT, BL], dt.bfloat16, tag="s")
            C_prev = sC_pool.tile([128, HT, BL], dt.bfloat16, tag="C")
            rs_prev = rs_pool.tile([128, HT, BL], dt.bfloat16, tag="rs")
            nc.vector.memset(s_prev, 0.0)
            nc.vector.memset(C_prev, 2.0)
            nc.vector.memset(rs_prev, 0.0)

            # prologue: chunk 0 currents
            sp = emit_sp_dma(0)
            io_cur = io_pool.tile([128, CH, HT, BL], dt.bfloat16, tag="io")
            for ih in range(HT):
                emit_prepass(sp, io_cur, 0, ih)
            io_next = None
            sp_next = None

            for t in range(cfg["nsteps"]):
                c, tin = t // CH, t % CH
                iin = io_cur[:, tin, :, :]

                # h, r updates (depend only on previous step state) — emitted
                # first so rs for step t+1 is produced as early as possible.
                # rs is ping-ponged so stage 1 below reads rs_{t-1} race-free.
                # States carry 2x scaling (s2 = 2*spike); the 0.5 is folded
                # into pout and Wout (exact powers of two).
                nc.vector.scalar_tensor_tensor(hs, hs, lr, s_prev,
                                               op0=A.mult, op1=A.add)
                rs_next = rs_pool.tile([128, HT, BL], dt.bfloat16, tag="rs")
                nc.vector.scalar_tensor_tensor(rs_next, rs_prev, ld, hs,
                                               op0=A.mult, op1=A.add)
                nc.vector.tensor_tensor(rms, rms, rs_next, op=A.max)

                # stage 1: u[pp,q,b] = sum_j poutT[j,q] rs_{t-1}[j]  (PSUM fp32)
                u_ps = u_pool.tile([128, PT, BL], dt.float32, tag="u")
                for q in range(PT):
                    for j in range(HT):
                        nc.tensor.matmul(u_ps[:, q, :], poutT[:, j, q, :],
                                         rs_prev[:, j, :],
                                         start=(j == 0), stop=(j == HT - 1))
                u_bf = ubf_pool.tile([128, PT, BL], dt.bfloat16, tag="ubf")
                nc.scalar.activation(u_bf, u_ps, AF.Copy)

                # stage 2 (+ Iin fold): X[.,i,.] = sum_q pinT[q,i] u_bf[q] + I@iin[i]
                x_ps = x_pool.tile([128, HT, BL], dt.float32, tag="x")
                for i in range(HT):
                    nc.tensor.matmul(x_ps[:, i, :], pinT[:, 0, i, :],
                                     u_bf[:, 0, :], start=True, stop=False)
                    nc.tensor.matmul(x_ps[:, i, :], pinT[:, 1, i, :],
                                     u_bf[:, 1, :], start=False, stop=False)
                    nc.tensor.matmul(x_ps[:, i, :], iden, iin[:, i, :],
                                     start=False, stop=True)

                # interleaved prepass for chunk c+1 (2 h-tiles per step), after
                # this step's scan matmuls so the PE services them first
                if c + 1 < nchunks and t < (c + 1) * CH:
                    if tin == 0:
                        sp_next = emit_sp_dma(c + 1)
                        io_next = io_pool.tile([128, CH, HT, BL], dt.bfloat16,
                                               tag="io")
                    for ih in (2 * tin, 2 * tin + 1):
                        emit_prepass(sp_next, io_next, c + 1, ih)

                # membrane: mem' = (C==0) * (lm*mem + X)
                mem_a = mema_pool.tile([128, HT, BL], dt.float32, tag="mema")
                nc.vector.scalar_tensor_tensor(mem_a, mem, lm, x_ps,
                                               op0=A.mult, op1=A.add)
                nc.vector.scalar_tensor_tensor(mem, C_prev, 0.0, mem_a,
                                               op0=A.is_equal, op1=A.mult)
                # s2 = 2*(mem > 1)
                s_new = sC_pool.tile([128, HT, BL], dt.bfloat16, tag="s")
                seng = nc.gpsimd if cfg["eng_s"] == "g" else nc.vector
                seng.tensor_scalar(s_new, mem, float(VTHR), 2.0,
                                   op0=A.is_gt, op1=A.mult)
                # D = relu(C-1);  C' = s2 + D
                D = sC_pool.tile([128, HT, BL], dt.bfloat16, tag="D")
                if cfg["eng_D"] == "g":
                    nc.gpsimd.tensor_scalar(D, C_prev, -1.0, 0.0,
                                            op0=A.add, op1=A.max)
                else:
                    nc.scalar.activation(D, C_prev, AF.Relu, bias=bm1, scale=1.0)
                C_new = sC_pool.tile([128, HT, BL], dt.bfloat16, tag="C")
                ceng = nc.gpsimd if cfg["eng_C"] == "g" else nc.vector
                ceng.tensor_tensor(C_new, s_new, D, op=A.add)
                s_prev, C_prev = s_new, C_new
                rs_prev = rs_next
                if tin == CH - 1 and io_next is not None:
                    io_cur = io_next
                    io_next = None

        # ---- readout: rout = (c*rms) @ Wout.T  (c folded into Wout) ----
        with tc.tile_pool(name="rp", bufs=1, space="PSUM") as rp_pool, \
             tc.tile_pool(name="ro", bufs=1) as ro_pool:
            rp = rp_pool.tile([BL, OUT], dt.float32, tag="rp")
            for j in range(HT):
                nc.tensor.matmul(rp, rms[:, j, :], woutT[:, j, :],
                                 start=(j == 0), stop=(j == HT - 1))
            ro = ro_pool.tile([BL, OUT], dt.float32, tag="ro")
            nc.vector.tensor_copy(ro, rp)
            nc.sync.dma_start(rout_d, ro)

    nc.compile()
    return nc


def _get_nc():
    if "nc" not in _CACHE:
        _CACHE["nc"] = _build_nc()
    return _CACHE["nc"]


def _make_spikes(inputs):
    """Bit-exact reference spikes: bernoulli(key(42), inputs, (T,B,IN))."""
    import jax
    cpu = jax.devices("cpu")[0]
    with jax.default_device(cpu):
        spk = jax.random.bernoulli(
            jax.random.key(42), jax.numpy.asarray(inputs), shape=(T, B, IN))
        return np.asarray(spk)


def prepare_in_maps(inputs, Win, pin, pout, l, Wout):
    lm, ld, lr, c = _decay_consts()
    one_m_lm = np.float32(1.0) - lm

    Win2 = (one_m_lm * Win).astype(np.float32)          # (H, IN)
    # states carry an exact 2x scale (s2 = 2*spike) -> fold 0.5 here
    pout2 = (np.float32(0.5) * c * pout).astype(np.float32)   # (H, P)
    pin2 = (one_m_lm * (l * pin)).astype(np.float32)          # (H, P)
    wout2 = (np.float32(0.5) * c * Wout).astype(np.float32)   # (OUT, H)

    # poutT[p, j, q, pp] = pout2[j*128+p, q*128+pp]
    poutT = np.ascontiguousarray(
        pout2.reshape(HT, 128, PT, 128).transpose(1, 0, 2, 3)).astype(BF16)
    # pinT[pp, q, i, p] = pin2[i*128+p, q*128+pp]
    pinT = np.ascontiguousarray(
        pin2.reshape(HT, 128, PT, 128).transpose(3, 2, 0, 1)).astype(BF16)
    # winT[ik, k, ih, hp] = Win2_padded[ih*128+hp, k*128+ik]
    winp = np.zeros((H, INP), np.float32)
    winp[:, :IN] = Win2
    winT = np.ascontiguousarray(
        winp.reshape(HT, 128, KT, 128).transpose(3, 2, 0, 1)).astype(BF16)
    idenm = np.eye(128, dtype=np.float32).astype(BF16)
    # woutT[hp, j, o] = wout2[o, j*128+hp]
    woutT = np.ascontiguousarray(
        wout2.T.reshape(HT, 128, OUT).transpose(1, 0, 2)).astype(BF16)

    spk = _make_spikes(inputs)                          # (T, B, IN) bool
    # spk_all[k, ik, t, b_global] = spk[t, b, k*128+ik] (zero-padded input dim)
    sp = np.zeros((INP, T, B), BF16)
    sp[:IN] = spk.transpose(2, 0, 1).astype(BF16)
    sp = sp.reshape(KT, 128, T, B)

    in_maps = []
    for cid in range(NCORES):
        in_maps.append({
            "spk": np.ascontiguousarray(sp[:, :, :, cid * BL:(cid + 1) * BL]),
            "poutT": poutT,
            "pinT": pinT,
            "winT": winT,
            "iden": idenm,
            "woutT": woutT,
        })
    return in_maps


def run_device(nc, in_maps):
    from concourse.bass_utils import run_bass_kernel_spmd
    res = run_bass_kernel_spmd(nc, in_maps, list(range(NCORES)))
    return np.concatenate([res.results[cid]["rout"] for cid in range(NCORES)],
                          axis=0)


def _softmax32(x):
    e = np.exp(x - x.max(axis=1, keepdims=True), dtype=np.float32)
    return (e / e.sum(axis=1, keepdims=True, dtype=np.float32)).astype(np.float32)


def kernel(inputs, Win, pin, pout, l, Wout):
    inputs = np.asarray(inputs, np.float32)
    Win = np.asarray(Win, np.float32)
    pin = np.asarray(pin, np.float32)
    pout = np.asarray(pout, np.float32)
    l = np.asarray(l, np.float32)
    Wout = np.asarray(Wout, np.float32)

    nc = _get_nc()
    in_maps = prepare_in_maps(inputs, Win, pin, pout, l, Wout)
    rout = run_device(nc, in_maps)                      # (512, 10) fp32
    return _softmax32(rout)
